# revision 1
# baseline (speedup 1.0000x reference)
"""ConvGRU Trainium2 kernel (nn_ConvRnn): B=4, T=8, C_in=C_out=64, H=W=96, 3x3 SAME.

Strategy:
- 8 cores = 4 samples x 2 height-halves. Bottom halves are row-flipped on the
  host (weights row-flipped too) so a single SPMD program serves all cores.
- No cross-core communication: each core computes a shrinking extended region
  R_t = 48 + 2*(7-t) rows so the halo needed by later steps is computed
  redundantly (avg +15% compute, zero sync).
- Convs are 9 shifted fp32r matmuls (K=128 channels = [x|h], M=out channels,
  N<=512 pixels) accumulating in PSUM. W padded to 98 with zero columns; zero
  rows come from host padding / zero-filled h tiles.
- Layout: channel c of x at partition c (0:64); h/r/rh/z/h_tilde at partition
  64+c so every 2-input DVE op has equal base partitions.
- Per step: rz-conv -> sigmoid (r into XRH in-place, z into Zt via cross-base
  ACT) -> rh=r*h in-place in XRH -> h-conv -> tanh -> d=h~-h, p=z*d (in-place
  in HT) -> h_new = h+p into next XH; h_new rows [2,50) DMA'd to output.
"""
import os
import numpy as np

import concourse.bacc as bacc
import concourse.tile as tile
from concourse import mybir

F32 = mybir.dt.float32
F32R = mybir.dt.float32r
AF = mybir.ActivationFunctionType
ALU = mybir.AluOpType

B, T, C, HW = 4, 8, 64, 96
W = 96
WP = 98          # padded width
WIN = 66         # rows per shard window
NCORES = 8
PSUM_ROWS = 10   # rows per PSUM chunk (2 banks)
MM_ROWS = 5      # rows per matmul (N = 480 <= 512)
DVE_ROWS = 16    # rows per DVE elementwise chunk
DMA_ROWS = 17    # rows per x-load DMA piece


def _r_of(t):
    return 48 + 2 * (7 - t)


def build_program():
    nc = bacc.Bacc("TRN2", target_bir_lowering=False, debug=False,
                   enable_asserts=False, num_devices=NCORES)
    xs_d = nc.dram_tensor("xs", [T, C, WIN, WP], F32R, kind="ExternalInput").ap()
    wrz_d = nc.dram_tensor("wrz", [9, 128, 128], F32R, kind="ExternalInput").ap()
    wh_d = nc.dram_tensor("wh", [9, 128, 64], F32R, kind="ExternalInput").ap()
    brz_d = nc.dram_tensor("brz", [128, 1], F32, kind="ExternalInput").ap()
    bh_d = nc.dram_tensor("bh", [64, 1], F32, kind="ExternalInput").ap()
    zer_d = nc.dram_tensor("zeros", [C, WIN, WP], F32R, kind="ExternalInput").ap()
    out_d = nc.dram_tensor("out", [T, C, 48, W], F32, kind="ExternalOutput").ap()

    with tile.TileContext(nc) as tc:
        with tc.tile_pool(name="persist", bufs=1) as pp, \
             tc.tile_pool(name="prz", bufs=2, space="PSUM") as prz, \
             tc.tile_pool(name="ph", bufs=2, space="PSUM") as ph:
            wrz_t = pp.tile([128, 9, 128], F32R, name="wrz")
            wh_t = pp.tile([128, 9, 64], F32R, name="wh")
            brz_t = pp.tile([128, 1], F32, name="brz")
            bh_t = pp.tile([64, 1], F32, name="bh")
            nc.sync.dma_start(out=wrz_t, in_=wrz_d.rearrange("t k m -> k t m"))
            nc.sync.dma_start(out=wh_t, in_=wh_d.rearrange("t k m -> k t m"))
            nc.sync.dma_start(out=brz_t, in_=brz_d)
            nc.sync.dma_start(out=bh_t, in_=bh_d)

            # Persistent double-buffered tiles (explicit, so zero-fill persists)
            xh = [pp.tile([128, WIN, WP], F32R, name=f"xh{i}") for i in range(2)]
            xrh = [pp.tile([128, 64, WP], F32R, name=f"xrh{i}") for i in range(2)]
            zt = pp.tile([128, 64, W], F32, name="zt")
            ht = pp.tile([128, 62, W], F32, name="ht")

            # one-time zero fills of h-parts (provides h0 = 0 and zero pads)
            for i in range(2):
                nc.gpsimd.dma_start(out=xh[i][64:128], in_=zer_d)
                nc.gpsimd.dma_start(out=xrh[i][64:128], in_=zer_d[:, 0:64, :])

            for t in range(T):
                R = _r_of(t)
                cur = xh[t % 2]
                nxt = xh[(t + 1) % 2]
                xr = xrh[t % 2]

                # ---- x loads (split into row pieces for DMA parallelism) ----
                # XH x-part rows [0, R+4)
                r0 = 0
                while r0 < R + 4:
                    r1 = min(r0 + DMA_ROWS, R + 4)
                    nc.sync.dma_start(out=cur[0:64, r0:r1, :],
                                      in_=xs_d[t, :, r0:r1, :])
                    r0 = r1
                # XRH x-part rows: XRH row j = XH row j+1; need XH rows [1, R+3)
                r0 = 0
                while r0 < R + 2:
                    r1 = min(r0 + DMA_ROWS, R + 2)
                    nc.gpsimd.dma_start(out=xr[0:64, r0:r1, :],
                                        in_=xs_d[t, :, r0 + 1:r1 + 1, :])
                    r0 = r1

                # ---- rz conv: output rows XH [1, 3+R) ----
                c0 = 1
                while c0 < 3 + R:
                    cr = min(PSUM_ROWS, 3 + R - c0)
                    hh = cr // 2  # rows per bank (cr is always even)
                    pt = prz.tile([128, 2, 512], F32, name="przt", tag="przt")
                    for tap in range(9):
                        di, dj = tap // 3, tap % 3
                        for s in range(2):
                            b0 = c0 + s * hh
                            nc.tensor.matmul(
                                pt[:, s, 0:hh * W],
                                wrz_t[:, tap, :],
                                cur[:, b0 + di - 1: b0 + di - 1 + hh, dj:dj + W],
                                start=(tap == 0), stop=(tap == 8))
                    # r -> XRH[64:128] rows (c0-1 ..), interior cols (fp32r)
                    nc.scalar.activation(xr[64:128, c0 - 1:c0 - 1 + cr, 1:97],
                                         pt[64:128, :, 0:hh * W], AF.Sigmoid,
                                         bias=brz_t[64:128])
                    # z -> Zt[64:128] rows (c0-1 ..) [cross-base ACT]
                    nc.scalar.activation(zt[64:128, c0 - 1:c0 - 1 + cr, :],
                                         pt[0:64, :, 0:hh * W], AF.Sigmoid,
                                         bias=brz_t[0:64])
                    c0 += cr

                # ---- rh = r * h in place in XRH (rows XH [1, 3+R)) ----
                c0 = 1
                while c0 < 3 + R:
                    cr = min(DVE_ROWS, 3 + R - c0)
                    nc.vector.tensor_tensor(
                        xr[64:128, c0 - 1:c0 - 1 + cr, 1:97],
                        xr[64:128, c0 - 1:c0 - 1 + cr, 1:97].bitcast(F32),
                        cur[64:128, c0:c0 + cr, 1:97].bitcast(F32),
                        op=ALU.mult)
                    c0 += cr

                # ---- h-tilde conv: output rows XH [2, 2+R) ----
                c0 = 2
                while c0 < 2 + R:
                    cr = min(PSUM_ROWS, 2 + R - c0)
                    hh = cr // 2
                    pt = ph.tile([128, 2, 512], F32, name="pht", tag="pht")
                    for tap in range(9):
                        di, dj = tap // 3, tap % 3
                        for s in range(2):
                            b0 = c0 + s * hh
                            # XRH row j = XH row j+1: XH row (b0+di-1) -> -1
                            nc.tensor.matmul(
                                pt[0:64, s, 0:hh * W],
                                wh_t[:, tap, :],
                                xr[:, b0 + di - 2: b0 + di - 2 + hh, dj:dj + W],
                                start=(tap == 0), stop=(tap == 8))
                    # tanh -> HT[64:128] rows (c0-2 ..) [cross-base ACT]
                    nc.scalar.activation(ht[64:128, c0 - 2:c0 - 2 + cr, :],
                                         pt[0:64, :, 0:hh * W], AF.Tanh, bias=bh_t)
                    c0 += cr

                # ---- elementwise update, rows XH [2, 2+R) ----
                c0 = 2
                while c0 < 2 + R:
                    cr = min(DVE_ROWS, 2 + R - c0)
                    hrows = slice(c0 - 2, c0 - 2 + cr)   # HT rows
                    zrows = slice(c0 - 1, c0 - 1 + cr)   # Zt rows
                    xrows = slice(c0, c0 + cr)           # XH rows
                    # d = h~ - h (in place in HT)
                    nc.vector.tensor_tensor(
                        ht[64:128, hrows, :], ht[64:128, hrows, :],
                        cur[64:128, xrows, 1:97].bitcast(F32), op=ALU.subtract)
                    # p = z * d (in place in HT)
                    nc.vector.tensor_tensor(
                        ht[64:128, hrows, :], zt[64:128, zrows, :],
                        ht[64:128, hrows, :], op=ALU.mult)
                    # h_new = h + p -> next XH (fp32r)
                    nc.vector.tensor_tensor(
                        nxt[64:128, xrows, 1:97],
                        cur[64:128, xrows, 1:97].bitcast(F32),
                        ht[64:128, hrows, :], op=ALU.add)
                    c0 += cr

                # ---- store owned rows [2, 50) ----
                for piece in range(2):
                    a = 2 + 24 * piece
                    eng = nc.sync if piece == 0 else nc.gpsimd
                    eng.dma_start(
                        out=out_d[t, :, 24 * piece:24 * piece + 24, :],
                        in_=nxt[64:128, a:a + 24, 1:97].bitcast(F32))
    nc.compile()
    return nc


_NC_CACHE = None


def _get_nc():
    global _NC_CACHE
    if _NC_CACHE is None:
        _NC_CACHE = build_program()
    return _NC_CACHE


def prep_core_inputs(x, w_r, b_r, w_z, b_z, w_h, b_h):
    """Host-side shard prep. Returns list of 8 in_maps."""
    x = np.asarray(x, np.float32)
    # padded x: rows 0..99 = global -2..97, cols 0..97 = global -1..96
    xp = np.zeros((B, T, C, 100, WP), np.float32)
    xp[:, :, :, 2:98, 1:97] = x

    w_rz = np.concatenate([np.asarray(w_z), np.asarray(w_r)], axis=0)  # [128,128,3,3]
    w_hh = np.asarray(w_h)                                             # [64,128,3,3]
    brz = np.concatenate([np.asarray(b_z), np.asarray(b_r)]).astype(np.float32)
    bh = np.asarray(b_h).astype(np.float32)

    packs = {}
    for flip in (0, 1):
        wrz_f = w_rz[:, :, ::-1, :] if flip else w_rz
        wh_f = w_hh[:, :, ::-1, :] if flip else w_hh
        # [9, K, M]: tap = di*3+dj, entry [k, m] = w[m, k, di, dj]
        packs[flip] = (
            np.ascontiguousarray(wrz_f.transpose(2, 3, 1, 0).reshape(9, 128, 128)),
            np.ascontiguousarray(wh_f.transpose(2, 3, 1, 0).reshape(9, 128, 64)),
        )

    zeros = np.zeros((C, WIN, WP), np.float32)
    in_maps = []
    for core in range(NCORES):
        b, flip = core // 2, core % 2
        if flip == 0:
            shard = xp[b, :, :, 0:66, :]
        else:
            shard = xp[b, :, :, 34:100, :][:, :, ::-1, :]
        wrz_p, wh_p = packs[flip]
        in_maps.append({
            "xs": np.ascontiguousarray(shard),
            "wrz": wrz_p, "wh": wh_p,
            "brz": brz.reshape(128, 1), "bh": bh.reshape(64, 1),
            "zeros": zeros,
        })
    return in_maps


def assemble_output(results):
    out = np.empty((B, T, C, HW, HW), np.float32)
    for core in range(NCORES):
        b, flip = core // 2, core % 2
        shard = results[core]["out"]          # [T, C, 48, 96]
        if flip == 0:
            out[b, :, :, 0:48, :] = shard
        else:
            out[b, :, :, 48:96, :] = shard[:, :, ::-1, :]
    return out.reshape(B * T, C, HW, HW)


def run_on_hw(inputs, trace=False):
    from concourse.bass_utils import run_bass_kernel_spmd
    nc = _get_nc()
    in_maps = prep_core_inputs(**inputs)
    res = run_bass_kernel_spmd(nc, in_maps, list(range(NCORES)), trace=trace)
    return assemble_output(res.results), res


def kernel(**inputs):
    out, _ = run_on_hw(inputs, trace=False)
    return out



# revision 4
# speedup vs baseline: 82.6245x; 82.6245x over previous
"""ConvGRU Trainium2 kernel (nn_ConvRnn): B=4, T=8, C_in=C_out=64, H=W=96, 3x3 SAME.

Strategy:
- 8 cores = 4 samples x 2 height-halves. Bottom halves are row-flipped on the
  host (weights row-flipped too) so a single SPMD program serves all cores.
- No cross-core communication: each core computes a shrinking extended region
  R_t = 48 + 2*(7-t) rows so the halo needed by later steps is computed
  redundantly (avg +15% compute, zero sync).
- Convs are 9 shifted bf16 matmuls (K=128 channels = [x|h], M=out channels,
  N<=512 pixels) accumulating in fp32 PSUM. Width padded to 98 with zero
  columns (host pad for x, one-time memset for h).
- Numerics: matmul operands (x, h, r*h, weights) are bf16; PSUM accumulation,
  activations, gate arithmetic and the recurrent state HF are fp32, so
  rounding does not accumulate in the state. Output is stored bf16 and
  upcast on the host (rel err ~2e-3 vs the 2e-2 gate).
- Layout: channel c of x at partition c (0:64); h/r/rh/z/h_tilde/HF at
  partition 64+c so every 2-input DVE op has equal base partitions.
- Per step: rz-conv -> sigmoid (r into XRH bf16, z into Zt f32 via cross-base
  ACT) -> rh=r*h in-place in XRH (bf16) -> h-conv -> tanh -> p=z*(h~-h) using
  f32 HF state -> h_new into HF (f32) and next XH (bf16); owned rows DMA'd
  to the bf16 output.
- build_program(rep=N) unrolls the whole pass N times (state re-zeroed per
  rep) so test.py can measure on-device time per pass by differencing.
"""
import numpy as np

import concourse.bacc as bacc
import concourse.tile as tile
from concourse import mybir

F32 = mybir.dt.float32
BF16 = mybir.dt.bfloat16
AF = mybir.ActivationFunctionType
ALU = mybir.AluOpType

B, T, C, HW = 4, 8, 64, 96
W = 96
WP = 98          # padded width
WIN = 66         # rows per shard window
NCORES = 8
PSUM_MAX = 10    # max rows per PSUM chunk (2 banks, 5 rows x 96 = 480 <= 512)
DVE_ROWS = 16    # rows per DVE elementwise chunk
DMA_ROWS = 33    # rows per x-load DMA piece


def _r_of(t):
    return 48 + 2 * (7 - t)


def _chunks(n, maxc=PSUM_MAX):
    """Balanced even chunk sizes <= maxc summing to n (n even)."""
    k = -(-n // maxc)
    base = (n // k) & ~1
    rem = n - base * k
    sizes = [base + 2] * (rem // 2) + [base] * (k - rem // 2)
    assert sum(sizes) == n and all(s % 2 == 0 and s <= maxc for s in sizes)
    return sizes


def build_program(rep=1):
    nc = bacc.Bacc("TRN2", target_bir_lowering=False, debug=False,
                   enable_asserts=False, num_devices=NCORES)
    xs_d = nc.dram_tensor("xs", [T, C, WIN, WP], BF16, kind="ExternalInput").ap()
    wrz_d = nc.dram_tensor("wrz", [9, 128, 128], BF16, kind="ExternalInput").ap()
    wh_d = nc.dram_tensor("wh", [9, 128, 64], BF16, kind="ExternalInput").ap()
    brz_d = nc.dram_tensor("brz", [128, 1], F32, kind="ExternalInput").ap()
    bh_d = nc.dram_tensor("bh", [64, 1], F32, kind="ExternalInput").ap()
    out_d = nc.dram_tensor("out", [T, C, 48, W], BF16, kind="ExternalOutput").ap()

    with tile.TileContext(nc) as tc:
        with tc.tile_pool(name="persist", bufs=1) as pp, \
             tc.tile_pool(name="prz", bufs=2, space="PSUM") as prz, \
             tc.tile_pool(name="ph", bufs=2, space="PSUM") as ph:
            wrz_t = pp.tile([128, 9, 128], BF16, name="wrz")
            wh_t = pp.tile([128, 9, 64], BF16, name="wh")
            brz_t = pp.tile([128, 1], F32, name="brz")
            bh_t = pp.tile([64, 1], F32, name="bh")

            # Persistent double-buffered tiles (explicit, so zero-fill persists)
            xh = [pp.tile([128, WIN, WP], BF16, name=f"xh{i}") for i in range(2)]
            xrh = [pp.tile([128, 64, WP], BF16, name=f"xrh{i}") for i in range(2)]
            zt = pp.tile([128, 64, W], F32, name="zt")
            ht = pp.tile([128, 62, W], F32, name="ht")
            hf = pp.tile([128, 64, W], F32, name="hf")  # f32 recurrent state

            # One-time zero fill: pad columns / never-written halo rows of the
            # h-parts stay zero for the whole program.
            for i in range(2):
                nc.gpsimd.memset(xh[i][64:128], 0.0)
                nc.vector.memset(xrh[i][64:128], 0.0)

            def one_pass():
                # Per-pass state reset: weights/biases (resident model state
                # in serving, but cheap to include) and h0 = 0.
                nc.sync.dma_start(out=wrz_t, in_=wrz_d.rearrange("t k m -> k t m"))
                nc.sync.dma_start(out=wh_t, in_=wh_d.rearrange("t k m -> k t m"))
                nc.sync.dma_start(out=brz_t, in_=brz_d)
                nc.sync.dma_start(out=bh_t, in_=bh_d)
                for i in range(2):
                    nc.gpsimd.memset(xh[i][64:128, 2:WIN, :], 0.0)
                nc.vector.memset(hf[64:128], 0.0)

                for t in range(T):
                    R = _r_of(t)
                    cur = xh[t % 2]
                    nxt = xh[(t + 1) % 2]
                    xr = xrh[t % 2]

                    # ---- x loads (split into row pieces for DMA parallelism) ----
                    # XH x-part rows [0, R+4)
                    r0 = 0
                    while r0 < R + 4:
                        r1 = min(r0 + DMA_ROWS, R + 4)
                        eng = nc.sync if r0 == 0 else nc.gpsimd
                        eng.dma_start(out=cur[0:64, r0:r1, :],
                                      in_=xs_d[t, :, r0:r1, :])
                        r0 = r1
                    # XRH x-part rows: XRH row j = XH row j+1; need XH rows [1, R+3)
                    r0 = 0
                    while r0 < R + 2:
                        r1 = min(r0 + DMA_ROWS, R + 2)
                        eng = nc.gpsimd if r0 == 0 else nc.sync
                        eng.dma_start(out=xr[0:64, r0:r1, :],
                                      in_=xs_d[t, :, r0 + 1:r1 + 1, :])
                        r0 = r1

                    # ---- rz conv: output rows XH [1, 3+R) ----
                    c0 = 1
                    for cr in _chunks(R + 2):
                        hh = cr // 2  # rows per bank
                        pt = prz.tile([128, 2, 512], F32, name="przt", tag="przt")
                        for tap in range(9):
                            di, dj = tap // 3, tap % 3
                            for s in range(2):
                                b0 = c0 + s * hh
                                nc.tensor.matmul(
                                    pt[:, s, 0:hh * W],
                                    wrz_t[:, tap, :],
                                    cur[:, b0 + di - 1: b0 + di - 1 + hh, dj:dj + W],
                                    start=(tap == 0), stop=(tap == 8))
                        # r -> XRH[64:128] rows (c0-1 ..), interior cols (bf16)
                        nc.scalar.activation(xr[64:128, c0 - 1:c0 - 1 + cr, 1:97],
                                             pt[64:128, :, 0:hh * W], AF.Sigmoid,
                                             bias=brz_t[64:128])
                        # z -> Zt[64:128] rows (c0-1 ..) [cross-base ACT]
                        nc.scalar.activation(zt[64:128, c0 - 1:c0 - 1 + cr, :],
                                             pt[0:64, :, 0:hh * W], AF.Sigmoid,
                                             bias=brz_t[0:64])
                        c0 += cr

                    # ---- rh = r * h in place in XRH (rows XH [1, 3+R)) ----
                    c0 = 1
                    while c0 < 3 + R:
                        cr = min(DVE_ROWS, 3 + R - c0)
                        nc.vector.tensor_tensor(
                            xr[64:128, c0 - 1:c0 - 1 + cr, 1:97],
                            xr[64:128, c0 - 1:c0 - 1 + cr, 1:97],
                            cur[64:128, c0:c0 + cr, 1:97],
                            op=ALU.mult)
                        c0 += cr

                    # ---- h-tilde conv: output rows XH [2, 2+R) ----
                    c0 = 2
                    for cr in _chunks(R):
                        hh = cr // 2
                        pt = ph.tile([128, 2, 512], F32, name="pht", tag="pht")
                        for tap in range(9):
                            di, dj = tap // 3, tap % 3
                            for s in range(2):
                                b0 = c0 + s * hh
                                # XRH row j = XH row j+1: XH row (b0+di-1) -> -1
                                nc.tensor.matmul(
                                    pt[0:64, s, 0:hh * W],
                                    wh_t[:, tap, :],
                                    xr[:, b0 + di - 2: b0 + di - 2 + hh, dj:dj + W],
                                    start=(tap == 0), stop=(tap == 8))
                        # tanh -> HT[64:128] rows (c0-2 ..) [cross-base ACT]
                        nc.scalar.activation(ht[64:128, c0 - 2:c0 - 2 + cr, :],
                                             pt[0:64, :, 0:hh * W], AF.Tanh, bias=bh_t)
                        c0 += cr

                    # ---- elementwise update, rows XH [2, 2+R) ----
                    # All gate math in f32 against the HF state; h_new written
                    # both to HF (f32, for the next step's update) and to the
                    # next XH h-part (bf16, for the next step's convs).
                    c0 = 2
                    while c0 < 2 + R:
                        cr = min(DVE_ROWS, 2 + R - c0)
                        srows = slice(c0 - 2, c0 - 2 + cr)   # HT/HF rows
                        zrows = slice(c0 - 1, c0 - 1 + cr)   # Zt rows
                        xrows = slice(c0, c0 + cr)           # XH rows
                        # d = h~ - h (in place in HT, f32)
                        nc.vector.tensor_tensor(
                            ht[64:128, srows, :], ht[64:128, srows, :],
                            hf[64:128, srows, :], op=ALU.subtract)
                        # p = z * d (in place in HT, f32)
                        nc.vector.tensor_tensor(
                            ht[64:128, srows, :], zt[64:128, zrows, :],
                            ht[64:128, srows, :], op=ALU.mult)
                        # h_new = h + p -> next XH h-part (bf16 out)
                        nc.vector.tensor_tensor(
                            nxt[64:128, xrows, 1:97],
                            hf[64:128, srows, :],
                            ht[64:128, srows, :], op=ALU.add)
                        # h_new -> HF (f32, in place; after the bf16 write)
                        nc.vector.tensor_tensor(
                            hf[64:128, srows, :],
                            hf[64:128, srows, :],
                            ht[64:128, srows, :], op=ALU.add)
                        c0 += cr

                    # ---- store owned rows [2, 50) ----
                    for piece in range(2):
                        a = 2 + 24 * piece
                        eng = nc.sync if piece == 0 else nc.gpsimd
                        eng.dma_start(
                            out=out_d[t, :, 24 * piece:24 * piece + 24, :],
                            in_=nxt[64:128, a:a + 24, 1:97])

            if rep == 1:
                one_pass()
            else:
                # Hardware loop: the body is traced once; every iteration is a
                # full independent pass (state re-zeroed at the top), so
                # wall(rep=K) - wall(rep=1) == (K-1) * on-device pass time.
                with tc.For_i(0, rep):
                    one_pass()
    nc.compile()
    return nc


_NC_CACHE = {}


def _get_nc(rep=1):
    if rep not in _NC_CACHE:
        _NC_CACHE[rep] = build_program(rep)
    return _NC_CACHE[rep]


def _bf16(a):
    import ml_dtypes
    return np.asarray(a, np.float32).astype(ml_dtypes.bfloat16)


def prep_core_inputs(x, w_r, b_r, w_z, b_z, w_h, b_h):
    """Host-side shard prep. Returns list of 8 in_maps."""
    import ml_dtypes
    x = np.asarray(x, np.float32)
    # padded x: rows 0..99 = global -2..97, cols 0..97 = global -1..96
    xp = np.zeros((B, T, C, 100, WP), ml_dtypes.bfloat16)
    xp[:, :, :, 2:98, 1:97] = x.astype(ml_dtypes.bfloat16)

    w_rz = np.concatenate([np.asarray(w_z), np.asarray(w_r)], axis=0)  # [128,128,3,3]
    w_hh = np.asarray(w_h)                                             # [64,128,3,3]
    brz = np.concatenate([np.asarray(b_z), np.asarray(b_r)]).astype(np.float32)
    bh = np.asarray(b_h).astype(np.float32)

    packs = {}
    for flip in (0, 1):
        wrz_f = w_rz[:, :, ::-1, :] if flip else w_rz
        wh_f = w_hh[:, :, ::-1, :] if flip else w_hh
        # [9, K, M]: tap = di*3+dj, entry [k, m] = w[m, k, di, dj]
        packs[flip] = (
            _bf16(np.ascontiguousarray(
                wrz_f.transpose(2, 3, 1, 0).reshape(9, 128, 128))),
            _bf16(np.ascontiguousarray(
                wh_f.transpose(2, 3, 1, 0).reshape(9, 128, 64))),
        )

    in_maps = []
    for core in range(NCORES):
        b, flip = core // 2, core % 2
        if flip == 0:
            shard = xp[b, :, :, 0:66, :]
        else:
            shard = xp[b, :, :, 34:100, :][:, :, ::-1, :]
        wrz_p, wh_p = packs[flip]
        in_maps.append({
            "xs": np.ascontiguousarray(shard),
            "wrz": wrz_p, "wh": wh_p,
            "brz": brz.reshape(128, 1), "bh": bh.reshape(64, 1),
        })
    return in_maps


def assemble_output(results):
    out = np.empty((B, T, C, HW, HW), np.float32)
    for core in range(NCORES):
        b, flip = core // 2, core % 2
        shard = np.asarray(results[core]["out"], np.float32)  # [T, C, 48, 96]
        if flip == 0:
            out[b, :, :, 0:48, :] = shard
        else:
            out[b, :, :, 48:96, :] = shard[:, :, ::-1, :]
    return out.reshape(B * T, C, HW, HW)


def run_on_hw(inputs, trace=False):
    from concourse.bass_utils import run_bass_kernel_spmd
    nc = _get_nc()
    in_maps = prep_core_inputs(**inputs)
    res = run_bass_kernel_spmd(nc, in_maps, list(range(NCORES)), trace=trace)
    return assemble_output(res.results), res


def kernel(**inputs):
    out, _ = run_on_hw(inputs, trace=False)
    return out


# revision 6
# speedup vs baseline: 208.5747x; 2.5244x over previous
"""ConvGRU Trainium2 kernel (nn_ConvRnn): B=4, T=8, C_in=C_out=64, H=W=96, 3x3 SAME.

Strategy:
- 8 cores = 4 samples x 2 height-halves. Bottom halves are row-flipped on the
  host (weights row-flipped too) so a single SPMD program serves all cores.
- No cross-core communication: each core computes a shrinking extended region
  R_t = 48 + 2*(7-t) rows so the halo needed by later steps is computed
  redundantly (avg +15% compute, zero sync).
- Convs are 9 shifted bf16 matmuls (K=128 channels = [x|h], M=out channels,
  N<=512 pixels) accumulating in fp32 PSUM. Width padded to 98 with zero
  columns (host pad for x, one-time memset for h).
- Numerics: matmul operands (x, h, r*h, weights) are bf16; PSUM accumulation,
  activations, gate arithmetic and the recurrent state HF are fp32, so
  rounding does not accumulate in the state. Output is stored bf16 and
  upcast on the host (rel err ~2e-3 vs the 2e-2 gate).
- Layout: channel c of x at partition c (0:64); h/r/rh/z/h_tilde/HF at
  partition 64+c so every 2-input DVE op has equal base partitions.
- Per step: rz-conv -> sigmoid (r into XRH bf16, z into Zt f32 via cross-base
  ACT) -> rh=r*h in-place in XRH (bf16) -> h-conv -> tanh -> p=z*(h~-h) using
  f32 HF state -> h_new into HF (f32) and next XH (bf16); owned rows DMA'd
  to the bf16 output.
- build_program(rep=N) unrolls the whole pass N times (state re-zeroed per
  rep) so test.py can measure on-device time per pass by differencing.
"""
import numpy as np

import concourse.bacc as bacc
import concourse.tile as tile
from concourse import mybir

F32 = mybir.dt.float32
BF16 = mybir.dt.bfloat16
AF = mybir.ActivationFunctionType
ALU = mybir.AluOpType

B, T, C, HW = 4, 8, 64, 96
W = 96
WP = 98          # padded width
WIN = 66         # rows per shard window
NCORES = 8
PSUM_MAX = 10    # max rows per PSUM chunk (2 banks, 5 rows x 96 = 480 <= 512)
DVE_ROWS = 16    # rows per DVE elementwise chunk
DMA_ROWS = 33    # rows per x-load DMA piece


def _r_of(t):
    return 48 + 2 * (7 - t)


def _chunks(n, maxc=PSUM_MAX):
    """Balanced even chunk sizes <= maxc summing to n (n even)."""
    k = -(-n // maxc)
    base = (n // k) & ~1
    rem = n - base * k
    sizes = [base + 2] * (rem // 2) + [base] * (k - rem // 2)
    assert sum(sizes) == n and all(s % 2 == 0 and s <= maxc for s in sizes)
    return sizes


def build_program(rep=1, unroll=False):
    nc = bacc.Bacc("TRN2", target_bir_lowering=False, debug=False,
                   enable_asserts=False, num_devices=NCORES)
    xs_d = nc.dram_tensor("xs", [T, C, WIN, WP], BF16, kind="ExternalInput").ap()
    wrz_d = nc.dram_tensor("wrz", [9, 128, 128], BF16, kind="ExternalInput").ap()
    wh_d = nc.dram_tensor("wh", [9, 128, 64], BF16, kind="ExternalInput").ap()
    brz_d = nc.dram_tensor("brz", [128, 1], F32, kind="ExternalInput").ap()
    bh_d = nc.dram_tensor("bh", [64, 1], F32, kind="ExternalInput").ap()
    out_d = nc.dram_tensor("out", [T, C, 48, W], BF16, kind="ExternalOutput").ap()

    with tile.TileContext(nc) as tc:
        with tc.tile_pool(name="persist", bufs=1) as pp, \
             tc.tile_pool(name="prz", bufs=2, space="PSUM") as prz, \
             tc.tile_pool(name="ph", bufs=2, space="PSUM") as ph:
            wrz_t = pp.tile([128, 9, 128], BF16, name="wrz")
            wh_t = pp.tile([128, 9, 64], BF16, name="wh")
            brz_t = pp.tile([128, 1], F32, name="brz")
            bh_t = pp.tile([64, 1], F32, name="bh")

            # Persistent double-buffered tiles (explicit, so zero-fill persists)
            xh = [pp.tile([128, WIN, WP], BF16, name=f"xh{i}") for i in range(2)]
            xrh = [pp.tile([128, 64, WP], BF16, name=f"xrh{i}") for i in range(2)]
            zt = pp.tile([128, 64, W], F32, name="zt")
            ht = pp.tile([128, 62, W], F32, name="ht")
            hf = pp.tile([128, 64, W], F32, name="hf")  # f32 recurrent state

            # One-time zero fill: pad columns / never-written halo rows of the
            # h-parts stay zero for the whole program.
            for i in range(2):
                nc.gpsimd.memset(xh[i][64:128], 0.0)
                nc.vector.memset(xrh[i][64:128], 0.0)

            def one_pass():
                # Per-pass state reset: weights/biases (resident model state
                # in serving, but cheap to include) and h0 = 0.
                nc.sync.dma_start(out=wrz_t, in_=wrz_d.rearrange("t k m -> k t m"))
                nc.sync.dma_start(out=wh_t, in_=wh_d.rearrange("t k m -> k t m"))
                nc.sync.dma_start(out=brz_t, in_=brz_d)
                nc.sync.dma_start(out=bh_t, in_=bh_d)
                for i in range(2):
                    nc.gpsimd.memset(xh[i][64:128, 2:WIN, :], 0.0)
                nc.vector.memset(hf[64:128], 0.0)

                for t in range(T):
                    R = _r_of(t)
                    cur = xh[t % 2]
                    nxt = xh[(t + 1) % 2]
                    xr = xrh[t % 2]

                    # ---- x loads (split into row pieces for DMA parallelism) ----
                    # XH x-part rows [0, R+4)
                    r0 = 0
                    while r0 < R + 4:
                        r1 = min(r0 + DMA_ROWS, R + 4)
                        eng = nc.sync if r0 == 0 else nc.gpsimd
                        eng.dma_start(out=cur[0:64, r0:r1, :],
                                      in_=xs_d[t, :, r0:r1, :])
                        r0 = r1
                    # XRH x-part rows: XRH row j = XH row j+1; need XH rows [1, R+3)
                    r0 = 0
                    while r0 < R + 2:
                        r1 = min(r0 + DMA_ROWS, R + 2)
                        eng = nc.gpsimd if r0 == 0 else nc.sync
                        eng.dma_start(out=xr[0:64, r0:r1, :],
                                      in_=xs_d[t, :, r0 + 1:r1 + 1, :])
                        r0 = r1

                    # ---- rz conv: output rows XH [1, 3+R) ----
                    c0 = 1
                    for cr in _chunks(R + 2):
                        hh = cr // 2  # rows per bank
                        pt = prz.tile([128, 2, 512], F32, name="przt", tag="przt")
                        for tap in range(9):
                            di, dj = tap // 3, tap % 3
                            for s in range(2):
                                b0 = c0 + s * hh
                                nc.tensor.matmul(
                                    pt[:, s, 0:hh * W],
                                    wrz_t[:, tap, :],
                                    cur[:, b0 + di - 1: b0 + di - 1 + hh, dj:dj + W],
                                    start=(tap == 0), stop=(tap == 8))
                        # r -> XRH[64:128] rows (c0-1 ..), interior cols (bf16)
                        nc.scalar.activation(xr[64:128, c0 - 1:c0 - 1 + cr, 1:97],
                                             pt[64:128, :, 0:hh * W], AF.Sigmoid,
                                             bias=brz_t[64:128])
                        # z -> Zt[64:128] rows (c0-1 ..) [cross-base ACT]
                        nc.scalar.activation(zt[64:128, c0 - 1:c0 - 1 + cr, :],
                                             pt[0:64, :, 0:hh * W], AF.Sigmoid,
                                             bias=brz_t[0:64])
                        c0 += cr

                    # ---- rh = r * h in place in XRH (rows XH [1, 3+R)) ----
                    c0 = 1
                    while c0 < 3 + R:
                        cr = min(DVE_ROWS, 3 + R - c0)
                        nc.vector.tensor_tensor(
                            xr[64:128, c0 - 1:c0 - 1 + cr, 1:97],
                            xr[64:128, c0 - 1:c0 - 1 + cr, 1:97],
                            cur[64:128, c0:c0 + cr, 1:97],
                            op=ALU.mult)
                        c0 += cr

                    # ---- h-tilde conv: output rows XH [2, 2+R) ----
                    c0 = 2
                    for cr in _chunks(R):
                        hh = cr // 2
                        pt = ph.tile([128, 2, 512], F32, name="pht", tag="pht")
                        for tap in range(9):
                            di, dj = tap // 3, tap % 3
                            for s in range(2):
                                b0 = c0 + s * hh
                                # XRH row j = XH row j+1: XH row (b0+di-1) -> -1
                                nc.tensor.matmul(
                                    pt[0:64, s, 0:hh * W],
                                    wh_t[:, tap, :],
                                    xr[:, b0 + di - 2: b0 + di - 2 + hh, dj:dj + W],
                                    start=(tap == 0), stop=(tap == 8))
                        # tanh -> HT[64:128] rows (c0-2 ..) [cross-base ACT]
                        nc.scalar.activation(ht[64:128, c0 - 2:c0 - 2 + cr, :],
                                             pt[0:64, :, 0:hh * W], AF.Tanh, bias=bh_t)
                        c0 += cr

                    # ---- elementwise update, rows XH [2, 2+R) ----
                    # All gate math in f32 against the HF state; h_new written
                    # both to HF (f32, for the next step's update) and to the
                    # next XH h-part (bf16, for the next step's convs).
                    c0 = 2
                    while c0 < 2 + R:
                        cr = min(DVE_ROWS, 2 + R - c0)
                        srows = slice(c0 - 2, c0 - 2 + cr)   # HT/HF rows
                        zrows = slice(c0 - 1, c0 - 1 + cr)   # Zt rows
                        xrows = slice(c0, c0 + cr)           # XH rows
                        # d = h~ - h (in place in HT, f32)
                        nc.vector.tensor_tensor(
                            ht[64:128, srows, :], ht[64:128, srows, :],
                            hf[64:128, srows, :], op=ALU.subtract)
                        # p = z * d (in place in HT, f32)
                        nc.vector.tensor_tensor(
                            ht[64:128, srows, :], zt[64:128, zrows, :],
                            ht[64:128, srows, :], op=ALU.mult)
                        # h_new = h + p -> next XH h-part (bf16 out)
                        nc.vector.tensor_tensor(
                            nxt[64:128, xrows, 1:97],
                            hf[64:128, srows, :],
                            ht[64:128, srows, :], op=ALU.add)
                        # h_new -> HF (f32, in place; after the bf16 write)
                        nc.vector.tensor_tensor(
                            hf[64:128, srows, :],
                            hf[64:128, srows, :],
                            ht[64:128, srows, :], op=ALU.add)
                        c0 += cr

                    # ---- store owned rows [2, 50) ----
                    for piece in range(2):
                        a = 2 + 24 * piece
                        eng = nc.sync if piece == 0 else nc.gpsimd
                        eng.dma_start(
                            out=out_d[t, :, 24 * piece:24 * piece + 24, :],
                            in_=nxt[64:128, a:a + 24, 1:97])

            if rep == 1:
                one_pass()
            elif unroll:
                for _ in range(rep):
                    one_pass()
            else:
                # Hardware loop: the body is traced once; every iteration is a
                # full independent pass (state re-zeroed at the top), so
                # wall(rep=K) - wall(rep=1) == (K-1) * on-device pass time.
                with tc.For_i(0, rep):
                    one_pass()
    nc.compile()
    return nc


_NC_CACHE = {}


def _get_nc(rep=1):
    if rep not in _NC_CACHE:
        _NC_CACHE[rep] = build_program(rep)
    return _NC_CACHE[rep]


def _bf16(a):
    import ml_dtypes
    return np.asarray(a, np.float32).astype(ml_dtypes.bfloat16)


def prep_core_inputs(x, w_r, b_r, w_z, b_z, w_h, b_h):
    """Host-side shard prep. Returns list of 8 in_maps."""
    import ml_dtypes
    x = np.asarray(x, np.float32)
    # padded x: rows 0..99 = global -2..97, cols 0..97 = global -1..96
    xp = np.zeros((B, T, C, 100, WP), ml_dtypes.bfloat16)
    xp[:, :, :, 2:98, 1:97] = x.astype(ml_dtypes.bfloat16)

    w_rz = np.concatenate([np.asarray(w_z), np.asarray(w_r)], axis=0)  # [128,128,3,3]
    w_hh = np.asarray(w_h)                                             # [64,128,3,3]
    brz = np.concatenate([np.asarray(b_z), np.asarray(b_r)]).astype(np.float32)
    bh = np.asarray(b_h).astype(np.float32)

    packs = {}
    for flip in (0, 1):
        wrz_f = w_rz[:, :, ::-1, :] if flip else w_rz
        wh_f = w_hh[:, :, ::-1, :] if flip else w_hh
        # [9, K, M]: tap = di*3+dj, entry [k, m] = w[m, k, di, dj]
        packs[flip] = (
            _bf16(np.ascontiguousarray(
                wrz_f.transpose(2, 3, 1, 0).reshape(9, 128, 128))),
            _bf16(np.ascontiguousarray(
                wh_f.transpose(2, 3, 1, 0).reshape(9, 128, 64))),
        )

    in_maps = []
    for core in range(NCORES):
        b, flip = core // 2, core % 2
        if flip == 0:
            shard = xp[b, :, :, 0:66, :]
        else:
            shard = xp[b, :, :, 34:100, :][:, :, ::-1, :]
        wrz_p, wh_p = packs[flip]
        in_maps.append({
            "xs": np.ascontiguousarray(shard),
            "wrz": wrz_p, "wh": wh_p,
            "brz": brz.reshape(128, 1), "bh": bh.reshape(64, 1),
        })
    return in_maps


def assemble_output(results):
    out = np.empty((B, T, C, HW, HW), np.float32)
    for core in range(NCORES):
        b, flip = core // 2, core % 2
        shard = np.asarray(results[core]["out"], np.float32)  # [T, C, 48, 96]
        if flip == 0:
            out[b, :, :, 0:48, :] = shard
        else:
            out[b, :, :, 48:96, :] = shard[:, :, ::-1, :]
    return out.reshape(B * T, C, HW, HW)


def run_on_hw(inputs, trace=False):
    from concourse.bass_utils import run_bass_kernel_spmd
    nc = _get_nc()
    in_maps = prep_core_inputs(**inputs)
    res = run_bass_kernel_spmd(nc, in_maps, list(range(NCORES)), trace=trace)
    return assemble_output(res.results), res


def kernel(**inputs):
    out, _ = run_on_hw(inputs, trace=False)
    return out
